# revision 1
# baseline (speedup 1.0000x reference)
"""Self-contained Trainium2 Bass kernel for nn_DbrxBlock_40492951667588.

DBRX block: LN1 -> GQA attention (RoPE, causal) -> residual+LN2 -> top-2/8 MoE.
8 NeuronCores, two SPMD launches:
  launch 1: token-parallel attention (core r owns batch-0 block r + batch-1
            block 7-r; causal kv sets balance to 1152 tokens/core).
  host:     router softmax/top-2 from device logits, capacity-padded dispatch.
  launch 2: expert-parallel MoE (core e owns expert e).
Matmuls run in float32r (TF32-like, ~1.5e-4 rel err); LN weights are folded
into adjacent matmul weights on the host (exact).
"""
import numpy as np
import concourse.bacc as bacc
import concourse.bass as bass
import concourse.mybir as mybir
import concourse.tile as tile
from concourse.bass_utils import run_bass_kernel_spmd

F32 = mybir.dt.float32
F32R = mybir.dt.float32r
AF = mybir.ActivationFunctionType

D = 2048
DT = D // 128          # 16 d-tiles
TKV = 1152             # kv tokens per core
NKT = TKV // 128       # 9 kv tiles
TQ = 256               # own q tokens
NH, KVH, HD = 16, 4, 128
NQB = 2
EPS = 1e-5
NEG = -30000.0

SCH = [(0, 384), (384, 384), (768, 384)]   # TKV chunks (psum-bank sized)


def bc_ap(ap, parts, n):
    """Partition-broadcast read AP: [parts, n] from a [1, n] row."""
    return bass.AP(tensor=ap.tensor, offset=ap.offset, ap=[[0, parts], [1, n]])


def build_attn(n_cores=8):
    nc = bacc.Bacc("TRN2", target_bir_lowering=False, debug=False,
                   num_devices=n_cores)
    xt = nc.dram_tensor("xt", [DT, 128, TKV], F32R, kind="ExternalInput").ap()
    wk = nc.dram_tensor("wk", [KVH, 128, DT, 128], F32R, kind="ExternalInput").ap()
    wv = nc.dram_tensor("wv", [128, DT, 512], F32R, kind="ExternalInput").ap()
    wq = nc.dram_tensor("wq", [NH, 128, DT, 128], F32R, kind="ExternalInput").ap()
    wo = nc.dram_tensor("wo", [DT, 128, DT, 128], F32R, kind="ExternalInput").ap()
    wr = nc.dram_tensor("wr", [128, DT, 8], F32R, kind="ExternalInput").ap()
    wksum = nc.dram_tensor("wksum", [128, KVH], F32, kind="ExternalInput").ap()
    wqsum = nc.dram_tensor("wqsum", [128, NH], F32, kind="ExternalInput").ap()
    wvsum = nc.dram_tensor("wvsum", [1, 512], F32, kind="ExternalInput").ap()
    cosk = nc.dram_tensor("cosk", [128, TKV], F32, kind="ExternalInput").ap()
    sink = nc.dram_tensor("sink", [128, TKV], F32, kind="ExternalInput").ap()
    cosq = nc.dram_tensor("cosq", [128, TQ], F32, kind="ExternalInput").ap()
    sinq = nc.dram_tensor("sinq", [128, TQ], F32, kind="ExternalInput").ap()
    masks = nc.dram_tensor("masks", [NQB, 128, TKV], F32, kind="ExternalInput").ap()
    ones = nc.dram_tensor("ones", [128, 1], F32R, kind="ExternalInput").ap()
    ident = nc.dram_tensor("ident", [128, 128], F32R, kind="ExternalInput").ap()

    rest = nc.dram_tensor("rest", [DT, 128, TQ], F32, kind="ExternalOutput").ap()
    h2t = nc.dram_tensor("h2t", [DT, 128, TQ], F32, kind="ExternalOutput").ap()
    logt = nc.dram_tensor("logt", [8, TQ], F32, kind="ExternalOutput").ap()

    scratch = nc.dram_tensor("scratch", [4, TKV], F32).ap()  # stat-row bounce

    with tile.TileContext(nc) as tc:
        with (
            tc.tile_pool(name="rows", bufs=1) as rows,
            tc.tile_pool(name="kvq", bufs=1) as kvq,
        ):
            ones_sb = rows.tile([128, 1], F32R)
            nc.sync.dma_start(out=ones_sb[:], in_=ones[:])
            ident_sb = rows.tile([128, 128], F32R)
            nc.sync.dma_start(out=ident_sb[:], in_=ident[:])
            wksum_sb = rows.tile([128, KVH], F32)
            nc.sync.dma_start(out=wksum_sb[:], in_=wksum[:])
            wqsum_sb = rows.tile([128, NH], F32)
            nc.sync.dma_start(out=wqsum_sb[:], in_=wqsum[:])
            wvsum_bc = rows.tile([128, 512], F32)
            nc.sync.dma_start(out=wvsum_bc[:], in_=bc_ap(wvsum, 128, 512))
            eps_t = rows.tile([1, 1], F32)
            nc.vector.memset(eps_t[:], EPS)

            kT = kvq.tile([128, KVH, TKV], F32R)
            vN = kvq.tile([128, NKT, 512], F32R)
            qT = kvq.tile([128, NH, TQ], F32R)
            xq_res = kvq.tile([128, DT, TQ], F32)

            with tc.tile_pool(name="norm", bufs=1) as norm:
                rstd_bc = norm.tile([128, TKV], F32)
                nmr_bc = norm.tile([128, TKV], F32)
                rstd_col = norm.tile([128, NKT], F32)
                nmr_col = norm.tile([128, NKT], F32)

                with tc.tile_pool(name="xp", bufs=1) as xp:
                    xts = xp.tile([128, DT, TKV], F32R)
                    for d in range(DT):
                        nc.sync.dma_start(out=xts[:, d, :], in_=xt[d])
                    xtf = xts[:].bitcast(F32)

                    # ---------------- LN1 stats ----------------
                    with (
                        tc.tile_pool(name="strow", bufs=1) as strow,
                        tc.tile_pool(name="sqp", bufs=2) as sqp,
                        tc.tile_pool(name="ps_st", bufs=1, space="PSUM") as ps_st,
                    ):
                        mu_row = strow.tile([1, TKV], F32)
                        sqm_row = strow.tile([1, TKV], F32)
                        t_row = strow.tile([1, TKV], F32)
                        psum_s = [ps_st.tile([1, w], F32, name=f"pss{i}",
                                             tag=f"pss{i}")
                                  for i, (_, w) in enumerate(SCH)]
                        psum_q = [ps_st.tile([1, w], F32, name=f"psq{i}",
                                             tag=f"psq{i}")
                                  for i, (_, w) in enumerate(SCH)]
                        for d in range(DT):
                            sq = sqp.tile([128, TKV], F32R, tag="sq")
                            nc.scalar.activation(sq[:], xtf[:, d, :], AF.Square)
                            for i, (c0, w) in enumerate(SCH):
                                nc.tensor.matmul(psum_s[i][:], ones_sb[:],
                                                 xts[:, d, c0:c0 + w],
                                                 start=(d == 0),
                                                 stop=(d == DT - 1))
                                nc.tensor.matmul(psum_q[i][:], ones_sb[:],
                                                 sq[:, c0:c0 + w],
                                                 start=(d == 0),
                                                 stop=(d == DT - 1))
                        for i, (c0, w) in enumerate(SCH):
                            nc.scalar.mul(mu_row[:, c0:c0 + w], psum_s[i][:],
                                          1.0 / D)
                            nc.scalar.mul(sqm_row[:, c0:c0 + w], psum_q[i][:],
                                          1.0 / D)
                        # var = E[x^2]-mu^2; rstd=1/sqrt(var+eps); nmr=-mu*rstd
                        nc.vector.tensor_mul(t_row[:], mu_row[:], mu_row[:])
                        nc.vector.tensor_sub(sqm_row[:], sqm_row[:], t_row[:])
                        nc.scalar.activation(sqm_row[:], sqm_row[:], AF.Sqrt,
                                             bias=eps_t[:])
                        nc.vector.reciprocal(sqm_row[:], sqm_row[:])
                        nc.vector.tensor_mul(t_row[:], mu_row[:], sqm_row[:])
                        nc.scalar.mul(t_row[:], t_row[:], -1.0)
                        nc.sync.dma_start(out=scratch[0:1, :], in_=sqm_row[:])
                        nc.sync.dma_start(out=scratch[1:2, :], in_=t_row[:])
                        nc.sync.dma_start(out=rstd_bc[:],
                                          in_=bc_ap(scratch[0:1, :], 128, TKV))
                        nc.sync.dma_start(out=nmr_bc[:],
                                          in_=bc_ap(scratch[1:2, :], 128, TKV))
                        nc.sync.dma_start(
                            out=rstd_col[:],
                            in_=scratch[0, :].rearrange("(t p) -> p t", p=128))
                        nc.sync.dma_start(
                            out=nmr_col[:],
                            in_=scratch[1, :].rearrange("(t p) -> p t", p=128))

                    # ---------------- K proj + rope ----------------
                    with (
                        tc.tile_pool(name="ckp", bufs=1) as ckp,
                        tc.tile_pool(name="wkp", bufs=2) as wkp,
                        tc.tile_pool(name="ktp", bufs=2) as ktp,
                        tc.tile_pool(name="kf1", bufs=2) as kf1,
                        tc.tile_pool(name="ps_k", bufs=2, space="PSUM") as ps_k,
                    ):
                        cosk_sb = ckp.tile([128, TKV], F32)
                        nc.sync.dma_start(out=cosk_sb[:], in_=cosk[:])
                        sink_sb = ckp.tile([128, TKV], F32)
                        nc.sync.dma_start(out=sink_sb[:], in_=sink[:])
                        for ok in range(KVH):
                            wk_sb = wkp.tile([128, DT, 128], F32R, tag="wk")
                            nc.sync.dma_start(out=wk_sb[:], in_=wk[ok])
                            psk = [ps_k.tile([128, w], F32, name=f"psk{i}",
                                             tag=f"psk{i}")
                                   for i, (_, w) in enumerate(SCH)]
                            for d in range(DT):
                                for i, (c0, w) in enumerate(SCH):
                                    nc.tensor.matmul(psk[i][:], wk_sb[:, d, :],
                                                     xts[:, d, c0:c0 + w],
                                                     start=(d == 0),
                                                     stop=(d == DT - 1))
                            ktmp = ktp.tile([128, TKV], F32, tag="ktmp")
                            krot = ktp.tile([128, TKV], F32, tag="krot")
                            for i, (c0, w) in enumerate(SCH):
                                t1 = kf1.tile([128, 384], F32, tag="kpf1")
                                nc.scalar.activation(
                                    t1[:, :w], nmr_bc[:, c0:c0 + w], AF.Copy,
                                    scale=wksum_sb[:, ok:ok + 1])
                                nc.vector.tensor_mul(ktmp[:, c0:c0 + w],
                                                     psk[i][:],
                                                     rstd_bc[:, c0:c0 + w])
                                nc.vector.tensor_add(ktmp[:, c0:c0 + w],
                                                     ktmp[:, c0:c0 + w],
                                                     t1[:, :w])
                            nc.sync.dma_start(out=krot[0:64, :],
                                              in_=ktmp[64:128, :])
                            nc.sync.dma_start(out=krot[64:128, :],
                                              in_=ktmp[0:64, :])
                            nc.vector.tensor_mul(ktmp[:], ktmp[:], cosk_sb[:])
                            nc.vector.tensor_mul(krot[:], krot[:], sink_sb[:])
                            nc.vector.tensor_add(kT[:, ok, :], ktmp[:], krot[:])

                    # ---------------- V proj (t-major) ----------------
                    with (
                        tc.tile_pool(name="wvp", bufs=1) as wvp,
                        tc.tile_pool(name="vf1", bufs=2) as vf1,
                        tc.tile_pool(name="ps_v", bufs=2, space="PSUM") as ps_v,
                    ):
                        wv_sb = wvp.tile([128, DT, 512], F32R)
                        nc.sync.dma_start(out=wv_sb[:], in_=wv[:])
                        for tv in range(NKT):
                            psv = ps_v.tile([128, 512], F32, tag="psv")
                            for d in range(DT):
                                nc.tensor.matmul(
                                    psv[:], xts[:, d, tv * 128:(tv + 1) * 128],
                                    wv_sb[:, d, :],
                                    start=(d == 0), stop=(d == DT - 1))
                            t1 = vf1.tile([128, 512], F32, tag="vpf1")
                            nc.scalar.activation(t1[:], wvsum_bc[:], AF.Copy,
                                                 scale=nmr_col[:, tv:tv + 1])
                            t2 = vf1.tile([128, 512], F32, tag="vpf2")
                            nc.vector.tensor_scalar_mul(
                                t2[:], in0=psv[:],
                                scalar1=rstd_col[:, tv:tv + 1])
                            nc.vector.tensor_add(vN[:, tv, :], t1[:], t2[:])

                    # ---------------- Q proj + rope ----------------
                    with (
                        tc.tile_pool(name="cqp", bufs=1) as cqp,
                        tc.tile_pool(name="wqp", bufs=3) as wqp,
                        tc.tile_pool(name="qtp", bufs=2) as qtp,
                        tc.tile_pool(name="ps_q", bufs=2, space="PSUM") as ps_q,
                    ):
                        cosq_sb = cqp.tile([128, TQ], F32)
                        nc.sync.dma_start(out=cosq_sb[:], in_=cosq[:])
                        sinq_sb = cqp.tile([128, TQ], F32)
                        nc.sync.dma_start(out=sinq_sb[:], in_=sinq[:])
                        for oq in range(NH):
                            wq_sb = wqp.tile([128, DT, 128], F32R, tag="wq")
                            nc.sync.dma_start(out=wq_sb[:], in_=wq[oq])
                            psq = ps_q.tile([128, TQ], F32, tag="psq")
                            for d in range(DT):
                                nc.tensor.matmul(psq[:], wq_sb[:, d, :],
                                                 xts[:, d, 0:TQ],
                                                 start=(d == 0),
                                                 stop=(d == DT - 1))
                            qtmp = qtp.tile([128, TQ], F32, tag="qtmp")
                            qrot = qtp.tile([128, TQ], F32, tag="qrot")
                            t1 = qtp.tile([128, TQ], F32, tag="qpf1")
                            nc.scalar.activation(t1[:], nmr_bc[:, 0:TQ],
                                                 AF.Copy,
                                                 scale=wqsum_sb[:, oq:oq + 1])
                            nc.vector.tensor_mul(qtmp[:], psq[:],
                                                 rstd_bc[:, 0:TQ])
                            nc.vector.tensor_add(qtmp[:], qtmp[:], t1[:])
                            nc.sync.dma_start(out=qrot[0:64, :],
                                              in_=qtmp[64:128, :])
                            nc.sync.dma_start(out=qrot[64:128, :],
                                              in_=qtmp[0:64, :])
                            nc.vector.tensor_mul(qtmp[:], qtmp[:], cosq_sb[:])
                            nc.vector.tensor_mul(qrot[:], qrot[:], sinq_sb[:])
                            nc.vector.tensor_add(qT[:, oq, :], qtmp[:], qrot[:])

                    # own-q raw x for the residual add (outlives xts)
                    nc.vector.tensor_copy(xq_res[:], xtf[:, :, 0:TQ])

            # ---------------- attention ----------------
            with tc.tile_pool(name="attp", bufs=1) as attp:
                attnT = attp.tile([128, NH, TQ], F32R)
                with (
                    tc.tile_pool(name="mkp", bufs=1) as mkp,
                    tc.tile_pool(name="scp", bufs=2) as scp,
                    tc.tile_pool(name="srp", bufs=2) as srp,
                    tc.tile_pool(name="ptsp", bufs=2) as ptsp,
                    tc.tile_pool(name="ps_s", bufs=1, space="PSUM") as ps_s,
                    tc.tile_pool(name="ps_t", bufs=2, space="PSUM") as ps_t,
                    tc.tile_pool(name="ps_a", bufs=2, space="PSUM") as ps_a,
                ):
                    mask_sb = mkp.tile([128, NQB, TKV], F32)
                    nc.sync.dma_start(out=mask_sb[:],
                                      in_=masks.rearrange("b p t -> p b t"))
                    for kvh in range(KVH):
                        for qb in range(NQB):
                            pns = []
                            for j in range(4):
                                h = kvh * 4 + j
                                s_sb = scp.tile([128, TKV], F32, tag=f"s{j}")
                                rs = srp.tile([128, 2], F32, tag=f"rs{j}")
                                for i, (c0, w) in enumerate(SCH):
                                    pss = ps_s.tile([128, w], F32,
                                                    name=f"pssc{i}",
                                                    tag=f"pssc{i}")
                                    nc.tensor.matmul(
                                        pss[:],
                                        qT[:, h, qb * 128:(qb + 1) * 128],
                                        kT[:, kvh, c0:c0 + w])
                                    nc.vector.tensor_add(
                                        s_sb[:, c0:c0 + w], pss[:],
                                        mask_sb[:, qb, c0:c0 + w])
                                nc.scalar.activation(s_sb[:], s_sb[:], AF.Exp,
                                                     accum_out=rs[:, 0:1])
                                nc.vector.reciprocal(rs[:, 1:2], rs[:, 0:1])
                                pn = scp.tile([128, TKV], F32R, tag=f"pn{j}")
                                nc.vector.tensor_scalar_mul(
                                    pn[:], in0=s_sb[:], scalar1=rs[:, 1:2])
                                pns.append(pn)
                            psa = ps_a.tile([128, 512], F32, tag="psa")
                            for kt in range(NKT):
                                ptp = ps_t.tile([128, 512], F32R, tag="ptp")
                                for j in range(4):
                                    nc.tensor.transpose(
                                        ptp[:, j * 128:(j + 1) * 128],
                                        pns[j][:, kt * 128:(kt + 1) * 128],
                                        ident_sb[:])
                                pts = ptsp.tile([128, 512], F32R, tag="pts")
                                nc.scalar.copy(pts[:], ptp[:].bitcast(F32))
                                nc.tensor.matmul(
                                    psa[:],
                                    vN[:, kt, kvh * 128:(kvh + 1) * 128],
                                    pts[:],
                                    start=(kt == 0), stop=(kt == NKT - 1))
                            nc.scalar.copy(
                                attnT[:, kvh * 4:(kvh + 1) * 4,
                                      qb * 128:(qb + 1) * 128],
                                psa[:].rearrange("p (j q) -> p j q", j=4))

                # ---------------- out-proj + residual + LN2 ----------------
                with (
                    tc.tile_pool(name="outp", bufs=1) as outp,
                    tc.tile_pool(name="wop", bufs=3) as wop,
                    tc.tile_pool(name="sq2p", bufs=2) as sq2p,
                    tc.tile_pool(name="ps_o", bufs=2, space="PSUM") as ps_o,
                    tc.tile_pool(name="ps_l2", bufs=1, space="PSUM") as ps_l2,
                ):
                    residT = outp.tile([128, DT, TQ], F32R)
                    h2s = outp.tile([128, DT, TQ], F32R)
                    ps2s = ps_l2.tile([1, TQ], F32, tag="ps2s")
                    ps2q = ps_l2.tile([1, TQ], F32, tag="ps2q")
                    for d2 in range(DT):
                        wo_sb = wop.tile([128, DT, 128], F32R, tag="wo")
                        nc.sync.dma_start(out=wo_sb[:], in_=wo[d2])
                        pso = ps_o.tile([128, TQ], F32, tag="pso")
                        for o in range(DT):
                            nc.tensor.matmul(pso[:], wo_sb[:, o, :],
                                             attnT[:, o, :],
                                             start=(o == 0), stop=(o == DT - 1))
                        nc.vector.tensor_add(residT[:, d2, :], pso[:],
                                             xq_res[:, d2, :])
                        nc.sync.dma_start(out=rest[d2],
                                          in_=residT[:, d2, :].bitcast(F32))
                        sq2 = sq2p.tile([128, TQ], F32R, tag="sq2")
                        nc.scalar.activation(sq2[:],
                                             residT[:, d2, :].bitcast(F32),
                                             AF.Square)
                        nc.tensor.matmul(ps2s[:], ones_sb[:], residT[:, d2, :],
                                         start=(d2 == 0), stop=(d2 == DT - 1))
                        nc.tensor.matmul(ps2q[:], ones_sb[:], sq2[:],
                                         start=(d2 == 0), stop=(d2 == DT - 1))
                    # LN2 rows
                    mu2 = outp.tile([1, TQ], F32)
                    sqm2 = outp.tile([1, TQ], F32)
                    t_r2 = outp.tile([1, TQ], F32)
                    nc.scalar.mul(mu2[:], ps2s[:], 1.0 / D)
                    nc.scalar.mul(sqm2[:], ps2q[:], 1.0 / D)
                    nc.vector.tensor_mul(t_r2[:], mu2[:], mu2[:])
                    nc.vector.tensor_sub(sqm2[:], sqm2[:], t_r2[:])
                    nc.scalar.activation(sqm2[:], sqm2[:], AF.Sqrt,
                                         bias=eps_t[:])
                    nc.vector.reciprocal(sqm2[:], sqm2[:])
                    nc.vector.tensor_mul(t_r2[:], mu2[:], sqm2[:])
                    nc.scalar.mul(t_r2[:], t_r2[:], -1.0)
                    nc.sync.dma_start(out=scratch[2:3, 0:TQ], in_=sqm2[:])
                    nc.sync.dma_start(out=scratch[3:4, 0:TQ], in_=t_r2[:])
                    rstd2_bc = outp.tile([128, TQ], F32)
                    nc.sync.dma_start(out=rstd2_bc[:],
                                      in_=bc_ap(scratch[2:3, 0:TQ], 128, TQ))
                    nmr2_bc = outp.tile([128, TQ], F32)
                    nc.sync.dma_start(out=nmr2_bc[:],
                                      in_=bc_ap(scratch[3:4, 0:TQ], 128, TQ))

                    # ---------------- h2 + router logits ----------------
                    with (
                        tc.tile_pool(name="wrp", bufs=1) as wrp,
                        tc.tile_pool(name="ps_r", bufs=1, space="PSUM") as ps_r,
                    ):
                        wr_sb = wrp.tile([128, DT, 8], F32R)
                        nc.sync.dma_start(out=wr_sb[:], in_=wr[:])
                        psl = ps_r.tile([8, TQ], F32, tag="psl")
                        for d2 in range(DT):
                            nc.vector.tensor_mul(h2s[:, d2, :],
                                                 residT[:, d2, :].bitcast(F32),
                                                 rstd2_bc[:])
                            nc.vector.tensor_add(h2s[:, d2, :],
                                                 h2s[:, d2, :].bitcast(F32),
                                                 nmr2_bc[:])
                            nc.sync.dma_start(out=h2t[d2],
                                              in_=h2s[:, d2, :].bitcast(F32))
                            nc.tensor.matmul(psl[:], wr_sb[:, d2, :],
                                             h2s[:, d2, :],
                                             start=(d2 == 0),
                                             stop=(d2 == DT - 1))
                        lo = outp.tile([8, TQ], F32)
                        nc.scalar.copy(lo[:], psl[:])
                        nc.sync.dma_start(out=logt[:], in_=lo[:])
    nc.compile()
    return nc


# ======================= host-side prep =======================

def core_colmap(r, NB=8, BLK=128):
    """(batch, pos) per column for core r. cols: [own qb0, own qb1, rest]."""
    b = []
    b += [(0, r * BLK + i) for i in range(BLK)]
    b += [(1, (NB - 1 - r) * BLK + i) for i in range(BLK)]
    for j in range(r):
        b += [(0, j * BLK + i) for i in range(BLK)]
    for j in range(NB - 1 - r):
        b += [(1, j * BLK + i) for i in range(BLK)]
    return b


def host_attn_inputs(x, cos, sin, ln1_w, w_qkv, w_out, w_router, ln2_w,
                     n_cores=8):
    """Per-core input maps for build_attn. x [B,S,D]; cos/sin [S,HD]."""
    B, S, Dm = x.shape
    NB, BLK = S // 128, 128
    wqkvT = (w_qkv * ln1_w[None, :]).T.astype(np.float32)      # [D, 3072]
    wqm = wqkvT[:, :NH * HD]                                    # [D, 2048] Q
    wkm = wqkvT[:, NH * HD:NH * HD + 512]                       # [D, 512] K
    wvm = wqkvT[:, NH * HD + 512:]                              # [D, 512] V
    w_outT = w_out.T.astype(np.float32)                         # [O, D]
    sinp = sin.copy()
    sinp[:, :HD // 2] *= -1.0
    scale = np.float32(1.0 / np.sqrt(HD))

    wk_in = np.ascontiguousarray(
        wkm.reshape(DT, 128, KVH, 128).transpose(2, 1, 0, 3))  # [ok, p, d, k]
    wv_in = np.ascontiguousarray(wvm.reshape(DT, 128, 512).transpose(1, 0, 2))
    wq_in = np.ascontiguousarray(
        wqm.reshape(DT, 128, NH, 128).transpose(2, 1, 0, 3))   # [oq, p, d, k]
    wo_in = np.ascontiguousarray(
        w_outT.reshape(DT, 128, DT, 128).transpose(2, 1, 0, 3))  # [d2, p, o, k]
    wr_in = np.ascontiguousarray(
        ((w_router * ln2_w[None, :]).T.astype(np.float32))
        .reshape(DT, 128, 8).transpose(1, 0, 2))               # [p, d, 8]
    wksum = np.ascontiguousarray(wkm.sum(0).reshape(KVH, 128).T)  # [128, KVH]
    wqsum = np.ascontiguousarray(wqm.sum(0).reshape(NH, 128).T)   # [128, NH]
    wvsum = np.ascontiguousarray(wvm.sum(0).reshape(1, 512))
    ident = np.eye(128, dtype=np.float32)
    ones_in = np.ones((128, 1), np.float32)

    maps = []
    for r in range(n_cores):
        cm = core_colmap(r, NB, BLK)
        bs = np.array([c[0] for c in cm])
        ps = np.array([c[1] for c in cm])
        xTc = np.ascontiguousarray(x[bs, ps, :].T)              # [D, TKV]
        ck = np.ascontiguousarray(cos[ps].T)                    # [HD, TKV]
        sk = np.ascontiguousarray(sinp[ps].T)
        cq = np.ascontiguousarray(cos[ps[:TQ]].T) * scale
        sq = np.ascontiguousarray(sinp[ps[:TQ]].T) * scale
        msk = np.full((NQB, 128, TKV), NEG, np.float32)
        for qb in range(NQB):
            qb_b = bs[qb * 128]
            qb_p = ps[qb * 128:(qb + 1) * 128]
            okm = (bs[None, :] == qb_b) & (ps[None, :] <= qb_p[:, None])
            msk[qb][okm] = 0.0
        maps.append({
            "xt": np.ascontiguousarray(xTc.reshape(DT, 128, TKV)),
            "wk": wk_in, "wv": wv_in, "wq": wq_in, "wo": wo_in, "wr": wr_in,
            "wksum": wksum, "wqsum": wqsum, "wvsum": wvsum,
            "cosk": ck, "sink": sk, "cosq": cq, "sinq": sq,
            "masks": msk, "ones": ones_in, "ident": ident,
        })
    return maps


def assemble_attn_outputs(results, n_cores=8, NB=8, BLK=128):
    """results: per-core dicts. Returns h2T_full [D,T], resid_full [D,T],
    logits [T, 8] in (batch, pos) token order."""
    T = 2 * NB * BLK
    h2T = np.zeros((D, T), np.float32)
    rT = np.zeros((D, T), np.float32)
    lg = np.zeros((T, 8), np.float32)
    for r in range(n_cores):
        cm = core_colmap(r, NB, BLK)
        toks = np.array([b * NB * BLK + p for b, p in cm[:TQ]])
        h2T[:, toks] = results[r]["h2t"].reshape(D, TQ)
        rT[:, toks] = results[r]["rest"].reshape(D, TQ)
        lg[toks] = results[r]["logt"].T
    return h2T, rT, lg

# ======================= MoE launch (expert parallel) =======================
MD, MF = 2048, 2048
DT_, FT = MD // 128, MF // 128

def chunks(C):
    # free-dim chunks <=512 (PSUM bank), prefer fewest chunks all >=256
    if C <= 512:
        return [(0, C)]
    if C <= 1024:
        h = (C // 2 + 31) // 32 * 32
        return [(0, h), (h, C - h)]
    return [(0, 512), (512, 512), (1024, C - 1024)]


def build_moe(C, n_cores=8):
    CH = chunks(C)
    nc = bacc.Bacc("TRN2", target_bir_lowering=False, debug=False,
                   num_devices=n_cores)
    xe = nc.dram_tensor("xe", [DT_, 128, C], F32R, kind="ExternalInput").ap()
    wg = nc.dram_tensor("wg", [FT, 128, DT_, 128], F32R, kind="ExternalInput").ap()
    wu = nc.dram_tensor("wu", [FT, 128, DT_, 128], F32R, kind="ExternalInput").ap()
    wd = nc.dram_tensor("wd", [DT_, 128, FT, 128], F32R, kind="ExternalInput").ap()
    wec = nc.dram_tensor("wec", [1, C], F32, kind="ExternalInput").ap()
    ye = nc.dram_tensor("ye", [DT_, 128, C], F32, kind="ExternalOutput").ap()

    with tile.TileContext(nc) as tc:
        with (
            tc.tile_pool(name="res", bufs=1) as res,
            tc.tile_pool(name="wp", bufs=3) as wp,
            tc.tile_pool(name="sg", bufs=3) as sgp,
            tc.tile_pool(name="yo", bufs=3) as yop,
        ):
            xsb = res.tile([128, DT_, C], F32R)
            for d in range(DT_):
                nc.sync.dma_start(out=xsb[:, d, :], in_=xe[d])
            webc = res.tile([128, C], F32)
            nc.sync.dma_start(
                out=webc[:],
                in_=bass.AP(tensor=wec.tensor, offset=wec.offset,
                            ap=[[0, 128], [1, C]]),
            )
            mT = res.tile([128, FT, C], F32R)

            # --- gate/up + silu*u -> mT ---
            with (
                tc.tile_pool(name="psgu", bufs=1, space="PSUM") as psg,
                tc.tile_pool(name="psy", bufs=2, space="PSUM") as psy,
            ):
                for f in range(FT):
                    pgs = [psg.tile([128, w], F32, name=f"pg{ci}", tag=f"pg{ci}")
                           for ci, (_, w) in enumerate(CH)]
                    pus = [psg.tile([128, w], F32, name=f"pu{ci}", tag=f"pu{ci}")
                           for ci, (_, w) in enumerate(CH)]
                    wgt = wp.tile([128, DT_, 128], F32R, tag="wg")
                    nc.sync.dma_start(out=wgt[:], in_=wg[f])
                    wut = wp.tile([128, DT_, 128], F32R, tag="wu")
                    nc.sync.dma_start(out=wut[:], in_=wu[f])
                    for d in range(DT_):
                        for ci, (c0, w) in enumerate(CH):
                            nc.tensor.matmul(pgs[ci][:], wgt[:, d, :],
                                             xsb[:, d, c0:c0 + w],
                                             start=(d == 0), stop=(d == DT_ - 1))
                        for ci, (c0, w) in enumerate(CH):
                            nc.tensor.matmul(pus[ci][:], wut[:, d, :],
                                             xsb[:, d, c0:c0 + w],
                                             start=(d == 0), stop=(d == DT_ - 1))
                    for ci, (c0, w) in enumerate(CH):
                        sg = sgp.tile([128, 512], F32, tag="sg")
                        nc.scalar.activation(sg[:, :w], pgs[ci][:],
                                             mybir.ActivationFunctionType.Silu)
                        nc.vector.tensor_mul(mT[:, f, c0:c0 + w], sg[:, :w],
                                             pus[ci][:])

                # --- down + combine-weight scale -> ye ---
                for d2 in range(DT_):
                    pys = [psy.tile([128, w], F32, name=f"py{ci}", tag=f"py{ci}")
                           for ci, (_, w) in enumerate(CH)]
                    wdt = wp.tile([128, FT, 128], F32R, tag="wd")
                    nc.sync.dma_start(out=wdt[:], in_=wd[d2])
                    for f in range(FT):
                        for ci, (c0, w) in enumerate(CH):
                            nc.tensor.matmul(pys[ci][:], wdt[:, f, :],
                                             mT[:, f, c0:c0 + w],
                                             start=(f == 0), stop=(f == FT - 1))
                    for ci, (c0, w) in enumerate(CH):
                        yt = yop.tile([128, 512], F32, tag="yt")
                        nc.vector.tensor_mul(yt[:, :w], pys[ci][:],
                                             webc[:, c0:c0 + w])
                        nc.sync.dma_start(out=ye[d2, :, c0:c0 + w], in_=yt[:, :w])
    nc.compile()
    return nc


def host_moe_inputs(h2T_full, assign, aw, C, w_gate_f, w_up_f, w_down):
    """Build per-core input maps. h2T_full [D, T]; assign/aw lists per expert."""
    E = len(assign)
    maps = []
    for e in range(E):
        n = len(assign[e])
        assert n <= C, f"expert {e} count {n} > capacity {C}"
        xeT = np.zeros((MD, C), np.float32)
        xeT[:, :n] = h2T_full[:, assign[e]]
        wec = np.zeros((1, C), np.float32)
        wec[0, :n] = aw[e]
        maps.append({
            "xe": np.ascontiguousarray(xeT.reshape(DT_, 128, C)),
            "wg": np.ascontiguousarray(
                w_gate_f[e].reshape(DT_, 128, FT, 128).transpose(2, 1, 0, 3)),
            "wu": np.ascontiguousarray(
                w_up_f[e].reshape(DT_, 128, FT, 128).transpose(2, 1, 0, 3)),
            "wd": np.ascontiguousarray(
                w_down[e].reshape(FT, 128, DT_, 128).transpose(2, 1, 0, 3)),
            "wec": wec,
        })
    return maps


# ======================= top-level kernel =======================
E, K_TOP = 8, 2
_cache = {}


def _routing(logits):
    lm = logits.max(1, keepdims=True)
    p = np.exp(logits - lm)
    p /= p.sum(1, keepdims=True)
    top_e = np.argsort(-p, 1)[:, :K_TOP]
    top_w = np.take_along_axis(p, top_e, 1)
    top_w = top_w / np.abs(top_w).sum(1, keepdims=True)
    flat_e = top_e.ravel()
    flat_t = np.repeat(np.arange(logits.shape[0]), K_TOP)
    flat_w = top_w.ravel()
    assign = [flat_t[flat_e == e] for e in range(E)]
    aw = [flat_w[flat_e == e] for e in range(E)]
    return assign, aw


def kernel(hidden_states, cos, sin, ln1_w, ln2_w, w_qkv, w_out,
           w_router, w_gate, w_up, w_down):
    hidden_states = np.asarray(hidden_states, np.float32)
    cos = np.asarray(cos, np.float32)
    sin = np.asarray(sin, np.float32)
    ln1_w = np.asarray(ln1_w, np.float32)
    ln2_w = np.asarray(ln2_w, np.float32)
    w_qkv = np.asarray(w_qkv, np.float32)
    w_out = np.asarray(w_out, np.float32)
    w_router = np.asarray(w_router, np.float32)
    w_gate = np.asarray(w_gate, np.float32)
    w_up = np.asarray(w_up, np.float32)
    w_down = np.asarray(w_down, np.float32)
    B, S, Dm = hidden_states.shape

    if "attn" not in _cache:
        _cache["attn"] = build_attn()
    maps = host_attn_inputs(hidden_states, cos, sin, ln1_w, w_qkv, w_out,
                            w_router, ln2_w)
    res1 = run_bass_kernel_spmd(_cache["attn"], maps, list(range(8)))
    h2T, rT, lg = assemble_attn_outputs(res1.results)

    assign, aw = _routing(lg)
    counts = [len(a) for a in assign]
    C = max(256, (max(counts) + 63) // 64 * 64)

    if ("moe", C) not in _cache:
        _cache[("moe", C)] = build_moe(C)
    w_gate_f = w_gate * ln2_w[None, :, None]
    w_up_f = w_up * ln2_w[None, :, None]
    maps2 = host_moe_inputs(h2T, assign, aw, C, w_gate_f, w_up_f, w_down)
    res2 = run_bass_kernel_spmd(_cache[("moe", C)], maps2, list(range(8)))

    T = B * S
    out_full = np.zeros((T, MD), np.float32)
    for e in range(E):
        ye = res2.results[e]["ye"].reshape(MD, C)
        n = counts[e]
        out_full[assign[e]] += ye[:, :n].T

    out = out_full.reshape(B, S, Dm)
    residual = rT.T.reshape(B, S, Dm)
    return out, residual



# revision 9
# speedup vs baseline: 1.4747x; 1.4747x over previous
"""Self-contained Trainium2 Bass kernel for nn_DbrxBlock_40492951667588.

DBRX block: LN1 -> GQA attention (RoPE, causal) -> residual+LN2 -> top-2/8 MoE.
8 NeuronCores, three SPMD launches (host resharding between launches is free):
  L1 "qkv":  token-parallel (core r owns 256 contiguous tokens of batch r//4),
             LN1 + QKV projection + RoPE, all fp16 operands, t-major matmuls.
  host:      redistributes K/V so each core sees its full causal prefix.
  L2 "attn": token-parallel attention with transposed scores ([kv,q] layout:
             no prob transposes, masking via 0/1 multiply, softmax
             normalization deferred until after PV), out-proj, residual (fp32),
             LN2 + h2.  Router logits/top-2 are computed on the host from the
             fp32 residual (routing must match the reference bit-for-bit in
             top-2 choice; fp16 attention keeps resid error ~1e-4).
  L3 "moe":  expert-parallel (core e owns expert e), capacity-padded, fp16
             weights/activations, fp32 PSUM accumulation.
"""
import numpy as np
import concourse.bacc as bacc
import concourse.bass as bass
import concourse.mybir as mybir
import concourse.tile as tile
from concourse.bass_utils import run_bass_kernel_spmd

F32 = mybir.dt.float32
F16 = mybir.dt.float16
AF = mybir.ActivationFunctionType
ALU = mybir.AluOpType

B, S, D = 2, 1024, 2048
NH, KVH, HD = 16, 4, 128
DT = D // 128            # 16 d-tiles
TQ = 256                 # own tokens per core
NJ = 8                   # kv tiles of 128 (full batch prefix)
E, K_TOP = 8, 2
EPS = 1e-5
MD, MF = 2048, 2048
FT = MF // 128


def bc_ap(ap, parts, n):
    """Partition-broadcast read AP: [parts, n] from a [1, n] row."""
    return bass.AP(tensor=ap.tensor, offset=ap.offset, ap=[[0, parts], [1, n]])


# ======================= launch 1: LN1 + QKV + RoPE =======================

def build_qkv(n_cores=8):
    nc = bacc.Bacc("TRN2", target_bir_lowering=False, debug=False,
                   num_devices=n_cores)
    xt = nc.dram_tensor("xt", [128, DT * TQ], F16, kind="ExternalInput").ap()
    wq = nc.dram_tensor("wq", [DT, 128, 2048], F16, kind="ExternalInput").ap()
    wkv = nc.dram_tensor("wkv", [DT, 128, 1024], F16, kind="ExternalInput").ap()
    qcs = nc.dram_tensor("qcs", [2, 128, 4096], F16, kind="ExternalInput").ap()
    kcs = nc.dram_tensor("kcs", [2, 128, 2048], F16, kind="ExternalInput").ap()
    ones = nc.dram_tensor("ones", [128, 1], F16, kind="ExternalInput").ap()

    qo = nc.dram_tensor("qo", [2, 128, 2048], F16, kind="ExternalOutput").ap()
    ko = nc.dram_tensor("ko", [2, 128, 512], F16, kind="ExternalOutput").ap()
    vo = nc.dram_tensor("vo", [2, 128, 512], F16, kind="ExternalOutput").ap()

    scr = nc.dram_tensor("scr", [2, TQ], F16).ap()

    with tile.TileContext(nc) as tc:
        with (
            tc.tile_pool(name="cst", bufs=1) as cst,
            tc.tile_pool(name="xp", bufs=1) as xp,
            tc.tile_pool(name="wp", bufs=1) as wp,
        ):
            ones_sb = cst.tile([128, 1], F16)
            nc.sync.dma_start(out=ones_sb[:], in_=ones[:])
            eps_t = cst.tile([1, 1], F32)
            nc.vector.memset(eps_t[:], EPS)

            xts = xp.tile([128, DT, TQ], F16)
            nc.sync.dma_start(out=xts[:].rearrange("p d t -> p (d t)"),
                              in_=xt[:])
            hT = xp.tile([128, DT, TQ], F16)
            wq_sb = wp.tile([128, DT, 2048], F16)
            for d in range(DT):
                nc.sync.dma_start(out=wq_sb[:, d, :], in_=wq[d])
            wkv_sb = wp.tile([128, DT, 1024], F16)
            for d in range(DT):
                nc.sync.dma_start(out=wkv_sb[:, d, :], in_=wkv[d])
            qcs_sb = wp.tile([128, 2, 4096], F16)
            nc.sync.dma_start(out=qcs_sb[:],
                              in_=qcs.rearrange("t p c -> p t c"))
            kcs_sb = wp.tile([128, 2, 2048], F16)
            nc.sync.dma_start(out=kcs_sb[:],
                              in_=kcs.rearrange("t p c -> p t c"))

            # ---------------- LN1 stats + h ----------------
            with (
                tc.tile_pool(name="st", bufs=1) as st,
                tc.tile_pool(name="sqp", bufs=2) as sqp,
                tc.tile_pool(name="ps_st", bufs=1, space="PSUM") as ps_st,
            ):
                ps_s = ps_st.tile([1, TQ], F32, name="pss", tag="pss")
                ps_q = ps_st.tile([1, TQ], F32, name="psq", tag="psq")
                for d in range(DT):
                    sq = sqp.tile([128, TQ], F16, tag="sq")
                    nc.scalar.activation(sq[:], xts[:, d, :], AF.Square)
                    nc.tensor.matmul(ps_s[:], ones_sb[:], xts[:, d, :],
                                     start=(d == 0), stop=(d == DT - 1))
                    nc.tensor.matmul(ps_q[:], ones_sb[:], sq[:],
                                     start=(d == 0), stop=(d == DT - 1))
                mu = st.tile([1, TQ], F32)
                msq = st.tile([1, TQ], F32)
                tr = st.tile([1, TQ], F32)
                nc.scalar.mul(mu[:], ps_s[:], 1.0 / D)
                nc.scalar.mul(msq[:], ps_q[:], 1.0 / D)
                nc.vector.tensor_mul(tr[:], mu[:], mu[:])
                nc.vector.tensor_sub(msq[:], msq[:], tr[:])
                nc.scalar.activation(msq[:], msq[:], AF.Sqrt, bias=eps_t[:])
                nc.vector.reciprocal(msq[:], msq[:])  # rstd
                mu16 = st.tile([1, TQ], F16)
                rstd16 = st.tile([1, TQ], F16)
                nc.vector.tensor_copy(mu16[:], mu[:])
                nc.vector.tensor_copy(rstd16[:], msq[:])
                nc.sync.dma_start(out=scr[0:1, :], in_=mu16[:])
                nc.sync.dma_start(out=scr[1:2, :], in_=rstd16[:])
                mu_bc = st.tile([128, TQ], F16)
                rstd_bc = st.tile([128, TQ], F16)
                nc.sync.dma_start(out=mu_bc[:], in_=bc_ap(scr[0:1, :], 128, TQ))
                nc.sync.dma_start(out=rstd_bc[:],
                                  in_=bc_ap(scr[1:2, :], 128, TQ))
                for d in range(DT):
                    t1 = sqp.tile([128, TQ], F16, tag="ht")
                    nc.vector.tensor_sub(t1[:], xts[:, d, :], mu_bc[:])
                    nc.vector.tensor_mul(hT[:, d, :], t1[:], rstd_bc[:])

            # ---------------- projections + rope (t-major) ----------------
            with (
                tc.tile_pool(name="rp", bufs=2) as rp,
                tc.tile_pool(name="ps_q", bufs=1, space="PSUM") as ps_qp,
                tc.tile_pool(name="ps_kv", bufs=1, space="PSUM") as ps_kvp,
            ):
                for tt in range(2):
                    psq = [ps_qp.tile([128, 512], F32, name=f"pq{c}",
                                      tag=f"pq{c}") for c in range(4)]
                    pskv = [ps_kvp.tile([128, 512], F32, name=f"pkv{c}",
                                        tag=f"pkv{c}") for c in range(2)]
                    for d in range(DT):
                        lh = hT[:, d, tt * 128:(tt + 1) * 128]
                        for c in range(4):
                            nc.tensor.matmul(psq[c][:], lh,
                                             wq_sb[:, d, c * 512:(c + 1) * 512],
                                             start=(d == 0), stop=(d == DT - 1))
                        for c in range(2):
                            nc.tensor.matmul(pskv[c][:], lh,
                                             wkv_sb[:, d,
                                                    c * 512:(c + 1) * 512],
                                             start=(d == 0), stop=(d == DT - 1))
                    # q rope
                    q_sb = rp.tile([128, 16, 2, 64], F16, tag="qsb")
                    for c in range(4):
                        nc.scalar.copy(
                            q_sb[:].rearrange("p h two f -> p (h two f)")
                            [:, c * 512:(c + 1) * 512], psq[c][:])
                    qf = q_sb[:].rearrange("p h two f -> p (h two f)")
                    t1 = rp.tile([128, 2048], F16, tag="qt1")
                    t2 = rp.tile([128, 16, 2, 64], F16, tag="qt2")
                    nc.vector.tensor_mul(t1[:], qf, qcs_sb[:, tt, 0:2048])
                    sview = qcs_sb[:, tt, 2048:4096].rearrange(
                        "p (h two f) -> p h two f", h=16, two=2)
                    nc.vector.tensor_mul(t2[:, :, 0, :], q_sb[:, :, 1, :],
                                         sview[:, :, 0, :])
                    nc.vector.tensor_mul(t2[:, :, 1, :], q_sb[:, :, 0, :],
                                         sview[:, :, 1, :])
                    qro = rp.tile([128, 2048], F16, tag="qro")
                    nc.vector.tensor_add(
                        qro[:], t1[:],
                        t2[:].rearrange("p h two f -> p (h two f)"))
                    nc.sync.dma_start(out=qo[tt], in_=qro[:])
                    # k rope (cols 0:512 of kv), v copy (cols 512:1024)
                    k_sb = rp.tile([128, 4, 2, 64], F16, tag="ksb")
                    nc.scalar.copy(
                        k_sb[:].rearrange("p h two f -> p (h two f)"),
                        pskv[0][:])
                    kf = k_sb[:].rearrange("p h two f -> p (h two f)")
                    kt1 = rp.tile([128, 512], F16, tag="kt1")
                    kt2 = rp.tile([128, 4, 2, 64], F16, tag="kt2")
                    nc.vector.tensor_mul(kt1[:], kf, kcs_sb[:, tt, 0:512])
                    ksv = kcs_sb[:, tt, 512:1024].rearrange(
                        "p (h two f) -> p h two f", h=4, two=2)
                    nc.vector.tensor_mul(kt2[:, :, 0, :], k_sb[:, :, 1, :],
                                         ksv[:, :, 0, :])
                    nc.vector.tensor_mul(kt2[:, :, 1, :], k_sb[:, :, 0, :],
                                         ksv[:, :, 1, :])
                    kro = rp.tile([128, 512], F16, tag="kro")
                    nc.vector.tensor_add(
                        kro[:], kt1[:],
                        kt2[:].rearrange("p h two f -> p (h two f)"))
                    nc.sync.dma_start(out=ko[tt], in_=kro[:])
                    v_sb = rp.tile([128, 512], F16, tag="vsb")
                    nc.scalar.copy(v_sb[:], pskv[1][:])
                    nc.sync.dma_start(out=vo[tt], in_=v_sb[:])
    nc.compile()
    return nc


# ======================= launch 2: attention =======================

def build_attn(n_cores=8):
    nc = bacc.Bacc("TRN2", target_bir_lowering=False, debug=False,
                   num_devices=n_cores)
    qT = nc.dram_tensor("qT", [128, NH * TQ], F16, kind="ExternalInput").ap()
    kT = nc.dram_tensor("kT", [128, KVH * 1024], F16,
                        kind="ExternalInput").ap()
    vN = nc.dram_tensor("vN", [128, NJ * 512], F16, kind="ExternalInput").ap()
    msk = nc.dram_tensor("msk", [128, NJ * 1024], F16,
                         kind="ExternalInput").ap()
    x_t = nc.dram_tensor("x_t", [2, 128, 2048], F32,
                         kind="ExternalInput").ap()
    wod = nc.dram_tensor("wod", [16, 128, 2048], F16,
                         kind="ExternalInput").ap()
    ones = nc.dram_tensor("ones", [128, 1], F16, kind="ExternalInput").ap()

    ro = nc.dram_tensor("ro", [2, 128, 2048], F32, kind="ExternalOutput").ap()
    h2o = nc.dram_tensor("h2o", [2, 128, 2048], F16,
                         kind="ExternalOutput").ap()

    scr = nc.dram_tensor("scr", [KVH, 1024], F32).ap()

    with tile.TileContext(nc) as tc:
        with (
            tc.tile_pool(name="cst", bufs=1) as cst,
            tc.tile_pool(name="inp", bufs=1) as inp,
            tc.tile_pool(name="wop", bufs=1) as wop,
            tc.tile_pool(name="att", bufs=1) as att,
        ):
            ones_sb = cst.tile([128, 1], F16)
            nc.sync.dma_start(out=ones_sb[:], in_=ones[:])
            eps_t = cst.tile([128, 1], F32)
            nc.vector.memset(eps_t[:], EPS)

            qT_sb = inp.tile([128, NH, TQ], F16)
            nc.sync.dma_start(out=qT_sb[:].rearrange("p h t -> p (h t)"),
                              in_=qT[:])
            kT_sb = inp.tile([128, KVH, 1024], F16)
            nc.sync.dma_start(out=kT_sb[:].rearrange("p h t -> p (h t)"),
                              in_=kT[:])
            vN_sb = inp.tile([128, NJ, 512], F16)
            nc.sync.dma_start(out=vN_sb[:].rearrange("p j c -> p (j c)"),
                              in_=vN[:])
            msk_sb = inp.tile([128, NJ, 1024], F16)
            nc.sync.dma_start(out=msk_sb[:].rearrange("p j c -> p (j c)"),
                              in_=msk[:])
            wod_sb = wop.tile([128, 16, 2048], F16)
            for ot in range(16):
                nc.sync.dma_start(out=wod_sb[:, ot, :], in_=wod[ot])
            x_sb = wop.tile([128, 2, 2048], F32)
            nc.sync.dma_start(out=x_sb[:],
                              in_=x_t.rearrange("t p c -> p t c"))

            attnT = att.tile([128, 16, TQ], F16)
            pvout = att.tile([128, KVH, 1024], F16)

            # ---------------- attention core ----------------
            with (
                tc.tile_pool(name="scp", bufs=2, space="PSUM") as scp,
                tc.tile_pool(name="pvp", bufs=1, space="PSUM") as pvp,
                tc.tile_pool(name="smp", bufs=1, space="PSUM") as smp,
                tc.tile_pool(name="prb", bufs=3) as prb,
                tc.tile_pool(name="drn", bufs=2) as drn,
            ):
                for kvh in range(KVH):
                    ps_pv = pvp.tile([128, 1024], F32, name="pv", tag="pv")
                    ps_sm = [smp.tile([1, 512], F32, name=f"sm{i}",
                                      tag=f"sm{i}") for i in range(2)]
                    for j in range(NJ):
                        ps_sc = scp.tile([128, 1024], F32, tag="sc")
                        lk = kT_sb[:, kvh, j * 128:(j + 1) * 128]
                        # start/stop once per PSUM bank (start clears the
                        # whole bank's has_written bits)
                        for h4 in range(4):
                            nc.tensor.matmul(
                                ps_sc[:, h4 * 256:(h4 + 1) * 256], lk,
                                qT_sb[:, kvh * 4 + h4, :],
                                start=(h4 % 2 == 0), stop=(h4 % 2 == 1))
                        probs = prb.tile([128, 1024], F16, tag="probs")
                        nc.scalar.activation(probs[:], ps_sc[:], AF.Exp)
                        nc.vector.tensor_mul(probs[:], probs[:],
                                             msk_sb[:, j, :])
                        for i in range(2):
                            nc.tensor.matmul(
                                ps_sm[i][:], ones_sb[:],
                                probs[:, i * 512:(i + 1) * 512],
                                start=(j == 0), stop=(j == NJ - 1))
                        lv = vN_sb[:, j, kvh * 128:(kvh + 1) * 128]
                        for h4 in range(4):
                            nc.tensor.matmul(
                                ps_pv[:, h4 * 256:(h4 + 1) * 256], lv,
                                probs[:, h4 * 256:(h4 + 1) * 256],
                                start=(j == 0 and h4 % 2 == 0),
                                stop=(j == NJ - 1 and h4 % 2 == 1))
                    # fast psum drains; normalization happens lazily below
                    nc.scalar.copy(pvout[:, kvh, :], ps_pv[:])
                    srow = drn.tile([1, 1024], F32, tag="srow")
                    for i in range(2):
                        nc.scalar.copy(srow[:, i * 512:(i + 1) * 512],
                                       ps_sm[i][:])
                    nc.vector.reciprocal(srow[:], srow[:])
                    nc.sync.dma_start(out=scr[kvh:kvh + 1, :], in_=srow[:])
                    rbc = drn.tile([128, 1024], F32, tag="rbc")
                    nc.sync.dma_start(out=rbc[:],
                                      in_=bc_ap(scr[kvh:kvh + 1, :], 128,
                                                1024))
                    for h4 in range(4):
                        nc.vector.tensor_mul(
                            attnT[:, kvh * 4 + h4, :],
                            pvout[:, kvh, h4 * 256:(h4 + 1) * 256],
                            rbc[:, h4 * 256:(h4 + 1) * 256])

            # ---------------- out-proj + residual + LN2 ----------------
            with (
                tc.tile_pool(name="ps_o", bufs=1, space="PSUM") as ps_op,
                tc.tile_pool(name="ob", bufs=2) as ob,
            ):
                for tt in range(2):
                    ps_o = [ps_op.tile([128, 512], F32, name=f"po{c}",
                                       tag=f"po{c}") for c in range(4)]
                    for ot in range(16):
                        la = attnT[:, ot, tt * 128:(tt + 1) * 128]
                        for c in range(4):
                            nc.tensor.matmul(
                                ps_o[c][:], la,
                                wod_sb[:, ot, c * 512:(c + 1) * 512],
                                start=(ot == 0), stop=(ot == 15))
                    resid = ob.tile([128, 2048], F32, tag="resid")
                    for c in range(4):
                        nc.vector.tensor_add(
                            resid[:, c * 512:(c + 1) * 512], ps_o[c][:],
                            x_sb[:, tt, c * 512:(c + 1) * 512])
                    nc.sync.dma_start(out=ro[tt], in_=resid[:])
                    junk = ob.tile([128, 2048], F16, tag="junk")
                    ssum = ob.tile([128, 1], F32, tag="ssum")
                    ssq = ob.tile([128, 1], F32, tag="ssq")
                    nc.scalar.activation(junk[:], resid[:], AF.Square,
                                         accum_out=ssq[:])
                    nc.scalar.activation(junk[:], resid[:], AF.Copy,
                                         accum_out=ssum[:])
                    mu = ob.tile([128, 1], F32, tag="mu")
                    var = ob.tile([128, 1], F32, tag="var")
                    t0 = ob.tile([128, 1], F32, tag="t0")
                    nc.vector.tensor_scalar_mul(mu[:], in0=ssum[:],
                                                scalar1=1.0 / D)
                    nc.vector.tensor_scalar_mul(var[:], in0=ssq[:],
                                                scalar1=1.0 / D)
                    nc.vector.tensor_mul(t0[:], mu[:], mu[:])
                    nc.vector.tensor_sub(var[:], var[:], t0[:])
                    nc.scalar.activation(var[:], var[:], AF.Sqrt,
                                         bias=eps_t[:])
                    nc.vector.reciprocal(var[:], var[:])  # rstd
                    h2 = ob.tile([128, 2048], F16, tag="h2")
                    nc.vector.tensor_scalar(
                        out=h2[:], in0=resid[:], scalar1=mu[:],
                        scalar2=var[:], op0=ALU.subtract, op1=ALU.mult)
                    nc.sync.dma_start(out=h2o[tt], in_=h2[:])
    nc.compile()
    return nc


# ======================= launch 3: MoE (expert parallel) =======================

def chunks(C):
    if C <= 512:
        return [(0, C)]
    h = (C // 2 + 31) // 32 * 32
    return [(0, h), (h, C - h)]


def build_moe(C, n_cores=8):
    CH = chunks(C)
    nc = bacc.Bacc("TRN2", target_bir_lowering=False, debug=False,
                   num_devices=n_cores)
    xe = nc.dram_tensor("xe", [DT, 128, C], F16, kind="ExternalInput").ap()
    wg = nc.dram_tensor("wg", [FT, 128, DT, 128], F16,
                        kind="ExternalInput").ap()
    wu = nc.dram_tensor("wu", [FT, 128, DT, 128], F16,
                        kind="ExternalInput").ap()
    wd = nc.dram_tensor("wd", [DT, 128, FT, 128], F16,
                        kind="ExternalInput").ap()
    wec = nc.dram_tensor("wec", [1, C], F32, kind="ExternalInput").ap()
    ye = nc.dram_tensor("ye", [DT, 128, C], F16, kind="ExternalOutput").ap()

    with tile.TileContext(nc) as tc:
        with (
            tc.tile_pool(name="res", bufs=1) as res,
            tc.tile_pool(name="wp", bufs=3) as wp,
            tc.tile_pool(name="sg", bufs=3) as sgp,
            tc.tile_pool(name="yo", bufs=3) as yop,
        ):
            xsb = res.tile([128, DT, C], F16)
            for d in range(DT):
                nc.sync.dma_start(out=xsb[:, d, :], in_=xe[d])
            webc = res.tile([128, C], F32)
            nc.sync.dma_start(
                out=webc[:],
                in_=bass.AP(tensor=wec.tensor, offset=wec.offset,
                            ap=[[0, 128], [1, C]]))
            mT = res.tile([128, FT, C], F16)

            with (
                tc.tile_pool(name="psgu", bufs=1, space="PSUM") as psg,
                tc.tile_pool(name="psy", bufs=2, space="PSUM") as psy,
            ):
                for f in range(FT):
                    pgs = [psg.tile([128, w], F32, name=f"pg{ci}",
                                    tag=f"pg{ci}")
                           for ci, (_, w) in enumerate(CH)]
                    pus = [psg.tile([128, w], F32, name=f"pu{ci}",
                                    tag=f"pu{ci}")
                           for ci, (_, w) in enumerate(CH)]
                    wgt = wp.tile([128, DT, 128], F16, tag="wg")
                    nc.sync.dma_start(out=wgt[:], in_=wg[f])
                    wut = wp.tile([128, DT, 128], F16, tag="wu")
                    nc.sync.dma_start(out=wut[:], in_=wu[f])
                    for d in range(DT):
                        for ci, (c0, w) in enumerate(CH):
                            nc.tensor.matmul(pgs[ci][:], wgt[:, d, :],
                                             xsb[:, d, c0:c0 + w],
                                             start=(d == 0),
                                             stop=(d == DT - 1))
                        for ci, (c0, w) in enumerate(CH):
                            nc.tensor.matmul(pus[ci][:], wut[:, d, :],
                                             xsb[:, d, c0:c0 + w],
                                             start=(d == 0),
                                             stop=(d == DT - 1))
                    for ci, (c0, w) in enumerate(CH):
                        sg = sgp.tile([128, 512], F16, tag="sg")
                        nc.scalar.activation(sg[:, :w], pgs[ci][:], AF.Silu)
                        nc.vector.tensor_mul(mT[:, f, c0:c0 + w], sg[:, :w],
                                             pus[ci][:])

                for d2 in range(DT):
                    pys = [psy.tile([128, w], F32, name=f"py{ci}",
                                    tag=f"py{ci}")
                           for ci, (_, w) in enumerate(CH)]
                    wdt = wp.tile([128, FT, 128], F16, tag="wd")
                    nc.sync.dma_start(out=wdt[:], in_=wd[d2])
                    for f in range(FT):
                        for ci, (c0, w) in enumerate(CH):
                            nc.tensor.matmul(pys[ci][:], wdt[:, f, :],
                                             mT[:, f, c0:c0 + w],
                                             start=(f == 0),
                                             stop=(f == FT - 1))
                    for ci, (c0, w) in enumerate(CH):
                        yt = yop.tile([128, 512], F16, tag="yt")
                        nc.vector.tensor_mul(yt[:, :w], pys[ci][:],
                                             webc[:, c0:c0 + w])
                        nc.sync.dma_start(out=ye[d2, :, c0:c0 + w],
                                          in_=yt[:, :w])
    nc.compile()
    return nc


# ======================= host-side prep =======================

def host_qkv_inputs(x, cos, sin, ln1_w, w_qkv, n_cores=8):
    wqkvT = (w_qkv * ln1_w[None, :]).T.astype(np.float32)  # [D, 3072]
    wq_in = np.ascontiguousarray(
        wqkvT[:, :2048].reshape(DT, 128, 2048)).astype(np.float16)
    wkv_in = np.ascontiguousarray(
        wqkvT[:, 2048:].reshape(DT, 128, 1024)).astype(np.float16)
    sinp = sin.copy()
    sinp[:, :HD // 2] *= -1.0
    scale = np.float32(1.0 / np.sqrt(HD))
    ones_in = np.ones((128, 1), np.float16)

    maps = []
    for r in range(n_cores):
        b, m = r // 4, r % 4
        sl = slice(256 * m, 256 * m + 256)
        xc = x[b, sl, :]                                   # [256, D]
        xt = np.ascontiguousarray(
            xc.reshape(256, DT, 128).transpose(2, 1, 0).reshape(128, DT * 256)
        ).astype(np.float16)
        qcs = np.zeros((2, 128, 4096), np.float16)
        kcs = np.zeros((2, 128, 2048), np.float16)
        for tt in range(2):
            pos = slice(256 * m + tt * 128, 256 * m + tt * 128 + 128)
            cq = cos[pos] * scale
            sq = sinp[pos] * scale
            qcs[tt, :, :2048] = np.tile(cq, (1, NH))
            qcs[tt, :, 2048:] = np.tile(sq, (1, NH))
            kcs[tt, :, :512] = np.tile(cos[pos], (1, KVH))
            kcs[tt, :, 512:1024] = np.tile(sinp[pos], (1, KVH))
        maps.append({"xt": xt, "wq": wq_in, "wkv": wkv_in, "qcs": qcs,
                     "kcs": kcs, "ones": ones_in})
    return maps


def host_attn_inputs(res1, x, w_out, n_cores=8):
    # assemble full K, V per batch from L1 outputs
    Kf = np.zeros((B, S, 512), np.float16)   # [b, pos, kvh*128+hd]
    Vf = np.zeros((B, S, 512), np.float16)
    Qf = np.zeros((B, S, 2048), np.float16)
    for r in range(n_cores):
        b, m = r // 4, r % 4
        for tt in range(2):
            rows = slice(256 * m + tt * 128, 256 * m + tt * 128 + 128)
            Kf[b, rows] = res1[r]["ko"][tt]
            Vf[b, rows] = res1[r]["vo"][tt]
            Qf[b, rows] = res1[r]["qo"][tt]
    wod_in = np.ascontiguousarray(
        w_out.T.reshape(16, 128, 2048)).astype(np.float16)
    ones_in = np.ones((128, 1), np.float16)
    maps = []
    for r in range(n_cores):
        b, m = r // 4, r % 4
        sl = slice(256 * m, 256 * m + 256)
        qT = np.ascontiguousarray(
            Qf[b, sl].reshape(256, NH, HD).transpose(2, 1, 0)
            .reshape(128, NH * 256)).astype(np.float16)
        kT = np.ascontiguousarray(
            Kf[b].reshape(S, KVH, HD).transpose(2, 1, 0)
            .reshape(128, KVH * S)).astype(np.float16)
        vN = np.ascontiguousarray(
            Vf[b].reshape(NJ, 128, 512).transpose(1, 0, 2)
            .reshape(128, NJ * 512)).astype(np.float16)
        kv_pos = np.arange(S).reshape(NJ, 128)           # [j, kv]
        q_pos = 256 * m + np.arange(256)                 # [qi]
        m01 = (kv_pos[:, :, None] <= q_pos[None, None, :])  # [j, kv, qi]
        msk = np.ascontiguousarray(
            np.tile(m01.astype(np.float16), (1, 1, 4))
            .transpose(1, 0, 2).reshape(128, NJ * 1024))
        x_t = np.ascontiguousarray(
            x[b, sl].reshape(2, 128, 2048)).astype(np.float32)
        maps.append({"qT": qT, "kT": kT, "vN": vN, "msk": msk, "x_t": x_t,
                     "wod": wod_in, "ones": ones_in})
    return maps


def assemble_attn_outputs(res2, n_cores=8):
    resid = np.zeros((B, S, D), np.float32)
    h2 = np.zeros((B * S, D), np.float16)
    for r in range(n_cores):
        b, m = r // 4, r % 4
        sl = slice(256 * m, 256 * m + 256)
        resid[b, sl] = res2[r]["ro"].reshape(256, D)
        h2[b * S + 256 * m: b * S + 256 * m + 256] = \
            res2[r]["h2o"].reshape(256, D)
    return resid, h2


def _routing(logits):
    lm = logits.max(1, keepdims=True)
    p = np.exp(logits - lm)
    p /= p.sum(1, keepdims=True)
    top_e = np.argsort(-p, 1)[:, :K_TOP]
    top_w = np.take_along_axis(p, top_e, 1)
    top_w = top_w / np.abs(top_w).sum(1, keepdims=True)
    flat_e = top_e.ravel()
    flat_t = np.repeat(np.arange(logits.shape[0]), K_TOP)
    flat_w = top_w.ravel()
    assign = [flat_t[flat_e == e] for e in range(E)]
    aw = [flat_w[flat_e == e] for e in range(E)]
    return assign, aw


def host_moe_inputs(h2_full, assign, aw, C, w_gate_f, w_up_f, w_down):
    maps = []
    for e in range(E):
        n = len(assign[e])
        assert n <= C, f"expert {e} count {n} > capacity {C}"
        xeT = np.zeros((MD, C), np.float16)
        xeT[:, :n] = h2_full[assign[e]].T
        wec = np.zeros((1, C), np.float32)
        wec[0, :n] = aw[e]
        maps.append({
            "xe": np.ascontiguousarray(xeT.reshape(DT, 128, C)),
            "wg": np.ascontiguousarray(
                w_gate_f[e].reshape(DT, 128, FT, 128)
                .transpose(2, 1, 0, 3)).astype(np.float16),
            "wu": np.ascontiguousarray(
                w_up_f[e].reshape(DT, 128, FT, 128)
                .transpose(2, 1, 0, 3)).astype(np.float16),
            "wd": np.ascontiguousarray(
                w_down[e].reshape(FT, 128, DT, 128)
                .transpose(2, 1, 0, 3)).astype(np.float16),
            "wec": wec,
        })
    return maps


# ======================= top-level kernel =======================
_cache = {}


def kernel(hidden_states, cos, sin, ln1_w, ln2_w, w_qkv, w_out,
           w_router, w_gate, w_up, w_down):
    x = np.asarray(hidden_states, np.float32)
    cos = np.asarray(cos, np.float32)
    sin = np.asarray(sin, np.float32)
    ln1_w = np.asarray(ln1_w, np.float32)
    ln2_w = np.asarray(ln2_w, np.float32)
    w_qkv = np.asarray(w_qkv, np.float32)
    w_out = np.asarray(w_out, np.float32)
    w_router = np.asarray(w_router, np.float32)
    w_gate = np.asarray(w_gate, np.float32)
    w_up = np.asarray(w_up, np.float32)
    w_down = np.asarray(w_down, np.float32)

    if "qkv" not in _cache:
        _cache["qkv"] = build_qkv()
    maps1 = host_qkv_inputs(x, cos, sin, ln1_w, w_qkv)
    res1 = run_bass_kernel_spmd(_cache["qkv"], maps1, list(range(8)))

    if "attn" not in _cache:
        _cache["attn"] = build_attn()
    maps2 = host_attn_inputs(res1.results, x, w_out)
    res2 = run_bass_kernel_spmd(_cache["attn"], maps2, list(range(8)))
    resid, h2_full = assemble_attn_outputs(res2.results)

    # routing from fp32 residual (host LN2 + router matmul, fp64)
    r64 = resid.reshape(-1, D).astype(np.float64)
    h2h = (r64 - r64.mean(1, keepdims=True)) / np.sqrt(
        r64.var(1, keepdims=True) + EPS) * ln2_w
    logits = h2h @ w_router.T.astype(np.float64)
    assign, aw = _routing(logits)
    counts = [len(a) for a in assign]
    C = max(256, (max(counts) + 31) // 32 * 32)

    if ("moe", C) not in _cache:
        _cache[("moe", C)] = build_moe(C)
    w_gate_f = w_gate * ln2_w[None, :, None]
    w_up_f = w_up * ln2_w[None, :, None]
    maps3 = host_moe_inputs(h2_full, assign, aw, C, w_gate_f, w_up_f, w_down)
    res3 = run_bass_kernel_spmd(_cache[("moe", C)], maps3, list(range(8)))

    T = B * S
    out_full = np.zeros((T, MD), np.float32)
    for e in range(E):
        ye = res3.results[e]["ye"].reshape(MD, C).astype(np.float32)
        n = counts[e]
        out_full[assign[e]] += ye[:, :n].T
    out = out_full.reshape(B, S, D)
    return out, resid
